# revision 46
# baseline (speedup 1.0000x reference)
"""8-core Trainium2 Bass kernel for nn_Attention_86079734546756.

Sharding: B=4 batches x 2 head-groups (8 heads each) -> 8 cores.
Per core (batch b, head-group g):
  - fp16 Q/K projections (transposed-out, channel-major) emitted in o-tile
    (head-pair) passes so attention on the first head pairs starts early
  - V projection in natural layout with an appended ones column (softmax
    denominators fall out of the PV matmul)
  - scores S^T[k,q] = K_h Q_h^T via fp16 matmuls, two heads packed into the
    128-row PE array (dh=64 contraction each, base_partition 0/64)
  - exp on ScalarE (PSUM->SBUF, bf16 out); mask multiply on VectorE
  - transposed PV: stationary = P^T tile [128k x 128q], moving = V[k, dh|1]
    -> PSUM [q, dh|den]; full 128-deep contraction (half the PE time of the
    natural orientation) and the denominator lands per-PARTITION, so
    normalization is a cheap reciprocal + tensor_scalar on VectorE
  - PE transpose (via host-fed identity) flips [q, c] back to channel-major
    for the output projection
  - output projection (bf16) -> partial y (bf16) summed on host (+ wo_b)
Item order is phased (head-pairs 0,1 for all q-chunks, then 2,3) so exp can
run continuously on ScalarE while remaining projections fill PE gaps.
A post-pass splits multi-wait instructions because this walrus build encodes
one sync wait per instruction.
No max-subtraction in softmax: |alpha| <~ 60 so exp stays in fp32 range,
and masked lanes are zeroed exactly by multiplying with (1-mask) after exp.
"""

import os
import numpy as np
import ml_dtypes

B, S, D, H = 4, 2048, 1024, 16
DH = D // H          # 64
P = 128
HPC = 8              # heads per core
OC = 512             # output features per core (head-group width)
NKT = S // P         # 16 k-tiles
NQC = S // 512       # 4 q-chunks
NOT = OC // P        # 4 o-tiles (head pairs)
NDT = D // P         # 8 d-tiles
N_CORES = 8

LAST_EXEC_NS = None
LAST_RESULTS = None

_BF16 = ml_dtypes.bfloat16


def _split_multi_waits(nc, mybir, max_waits: int = 1):
    """The walrus build in this container encodes at most one sync wait per
    ISA instruction and refuses to split. Move extra waits onto standalone
    EventSemaphore instructions inserted just before, on the same engine —
    the engine executes them in stream order, so semantics are unchanged
    (DMA triggers are simply enqueued after the waits pass)."""
    ctr = 0
    for fn in nc.m.functions:
        for blk in fn.blocks:
            insts = blk.instructions
            if not any(
                inst.sync_info is not None
                and inst.sync_info.on_wait
                and len(inst.sync_info.on_wait) > max_waits
                for inst in insts
            ):
                continue
            out = []
            for inst in insts:
                si = inst.sync_info
                waits = list(si.on_wait) if si is not None and si.on_wait else []
                if len(waits) > max_waits:
                    extra, keep = waits[:-max_waits], waits[-max_waits:]
                    for w in extra:
                        ev = mybir.InstEventSemaphore(
                            name=f"evsplit-{ctr}",
                            engine=inst.engine,
                            ins=[],
                            outs=[],
                            sync_info=mybir.SyncInfo(on_wait=[w], on_update=[]),
                        )
                        ctr += 1
                        out.append(ev)
                    si.on_wait = keep
                out.append(inst)
            blk.instructions = out
    return ctr


def _build_program(with_qkv_bias: bool, split_waits: bool = True):
    from contextlib import ExitStack
    import concourse.bass as bass
    import concourse.mybir as mybir
    import concourse.tile as tile

    dt = mybir.dt
    AF = mybir.ActivationFunctionType
    ALU = mybir.AluOpType

    nc = bass.Bass(trn_type="TRN2")

    xq = nc.declare_dram_parameter("xq_t", [D, S], dt.float16, isOutput=False)
    xk = nc.declare_dram_parameter("xk_t", [D, S], dt.float16, isOutput=False)
    xv = nc.declare_dram_parameter("xv_t", [D, S], dt.float16, isOutput=False)
    invm = nc.declare_dram_parameter("invm_t", [S, S], dt.bfloat16, isOutput=False)
    wq = nc.declare_dram_parameter("wq_t", [D, OC], dt.float16, isOutput=False)
    wk = nc.declare_dram_parameter("wk_t", [D, OC], dt.float16, isOutput=False)
    wv = nc.declare_dram_parameter("wv_t", [D, OC], dt.float16, isOutput=False)
    wo = nc.declare_dram_parameter("wo_t", [OC, D], dt.bfloat16, isOutput=False)
    ident = nc.declare_dram_parameter("ident", [P, P], dt.bfloat16, isOutput=False)
    if with_qkv_bias:
        bq = nc.declare_dram_parameter("bq", [OC], dt.float32, isOutput=False)
        bk = nc.declare_dram_parameter("bk", [OC], dt.float32, isOutput=False)
        bv = nc.declare_dram_parameter("bv_bcast", [P, OC], dt.float32, isOutput=False)
    y = nc.declare_dram_parameter("y_part", [S, D], dt.bfloat16, isOutput=True)

    with tile.TileContext(nc) as tc, ExitStack() as ctx:
        persist = ctx.enter_context(tc.tile_pool(name="persist", bufs=1))
        wqkp = ctx.enter_context(tc.tile_pool(name="wqkp", bufs=2))
        wvp = ctx.enter_context(tc.tile_pool(name="wvp", bufs=1))
        xpool = ctx.enter_context(tc.tile_pool(name="xpool", bufs=6))
        invp = ctx.enter_context(tc.tile_pool(name="invp", bufs=4))
        ptp = ctx.enter_context(tc.tile_pool(name="ptp", bufs=2))
        ottp = ctx.enter_context(tc.tile_pool(name="ottp", bufs=2))
        rcp = ctx.enter_context(tc.tile_pool(name="rcp", bufs=8))
        yp = ctx.enter_context(tc.tile_pool(name="yp", bufs=2))
        scp = ctx.enter_context(tc.tile_pool(name="scp", bufs=2, space="PSUM"))
        mmp = ctx.enter_context(tc.tile_pool(name="mmp", bufs=3, space="PSUM"))
        pjp = ctx.enter_context(tc.tile_pool(name="pjp", bufs=1, space="PSUM"))

        QHT = persist.tile([P, NOT, S], dt.float16)          # [o%128, ot, s]
        KHT = persist.tile([P, NOT, S], dt.float16)
        VSB = persist.tile([P, NKT, HPC, DH + 1], dt.bfloat16)  # [k%128, kt, h, dh|1]
        OT = persist.tile([P, NOT, S], dt.bfloat16)          # [c%128, ct, s]
        WO = persist.tile([P, NOT, D], dt.bfloat16)          # [c%128, ct, o]
        IDT = persist.tile([P, P], dt.bfloat16)

        def late_init():
            # Deferred so the first projection's weight/x DMAs go out first.
            nc.sync.dma_start(IDT[:], ident[:])
            nc.vector.memset(VSB[:, :, :, DH : DH + 1], 1.0)

        if with_qkv_bias:
            bq_sb = persist.tile([P, NOT], dt.float32)
            nc.sync.dma_start(bq_sb[:], bq.rearrange("(ot p) -> p ot", p=P))
            bk_sb = persist.tile([P, NOT], dt.float32)
            nc.sync.dma_start(bk_sb[:], bk.rearrange("(ot p) -> p ot", p=P))
            bv_sb = persist.tile([P, OC], dt.float32)
            nc.sync.dma_start(bv_sb[:], bv[:])

        # ---------------- projections -----------------
        xqr = xq.rearrange("(dt p) s -> p dt s", p=P)
        xkr = xk.rearrange("(dt p) s -> p dt s", p=P)
        xvr = xv.rearrange("(dt p) s -> p dt s", p=P)

        def proj_pair_start(w_dram, op):
            """DMA the weights for o-tile pair (2*op, 2*op+1)."""
            wsb = wqkp.tile([P, NDT, 2 * P], dt.float16, tag="wqk", name="wsb")
            wr = w_dram.rearrange("(dt p) o -> p dt o", p=P)
            nc.sync.dma_start(wsb[:], wr[:, :, 2 * op * P : (2 * op + 2) * P])
            return wsb

        def proj_pair_sc(xr, wsb, dst, bias_sb, op, sc, dma=None):
            """One 512-token chunk of the projection for o-tile pair op.
            Loads all 8 d-tiles of x in a single DMA (the SP sequencer costs
            ~565ns per DMA instruction, so small per-dtile DMAs starve PE)."""
            # x loads in two 4-dtile halves so DMA triggers pipeline finely
            # (one 8-dtile load per chunk left the PE idle waiting the tail
            # of a 1MB transfer, which also resets the PE p-state). Alloc
            # PSUM after the DMAs: the scp-slot wait must not attach to the
            # DMA trigger.
            sl = slice(sc * 512, (sc + 1) * 512)
            xts = []
            for h in range(2):
                xh = xpool.tile([P, NDT // 2, 512], dt.float16, tag="x", name="xt")
                (dma or nc.sync).dma_start(xh[:], xr[:, h * 4 : h * 4 + 4, sl])
                xts.append(xh)
            ps = scp.tile([P, 2, 512], dt.float32, tag="sc", name="pjp")
            # k outer: one PSUM accumulation group fully closes before the
            # next slot's group opens (interleaved open groups in one tile
            # are rejected by the simulator's zero-region model).
            for k in range(2):
                for dti in range(NDT):
                    nc.tensor.matmul(
                        ps[:, k, :],
                        lhsT=wsb[:, dti, k * P : (k + 1) * P],
                        rhs=xts[dti // 4][:, dti % 4, :],
                        start=(dti == 0),
                        stop=(dti == NDT - 1),
                    )
            for k in range(2):
                ot = 2 * op + k
                dstap = dst[:, ot, sc * 512 : (sc + 1) * 512]
                if bias_sb is not None:
                    nc.vector.tensor_scalar(
                        dstap, ps[:, k, :], bias_sb[:, ot : ot + 1], None, ALU.add
                    )
                else:
                    nc.vector.tensor_copy(dstap, ps[:, k, :])

        def proj_pair(xr, w_dram, dst, bias_sb, op):
            wsb = proj_pair_start(w_dram, op)
            for sc in range(NQC):
                proj_pair_sc(xr, wsb, dst, bias_sb, op, sc)

        def emit_vproj():
            # V: natural layout [s, o] scattered into VSB[k%128, kt, h, 0:64].
            wvsb = wvp.tile([P, NDT, OC], dt.float16, tag="wv", name="wvsb")
            nc.sync.dma_start(wvsb[:], wv.rearrange("(dt p) o -> p dt o", p=P))
            for sc in range(NQC):
                xt = xpool.tile([P, NDT, 512], dt.float16, tag="x", name="xtv")
                nc.sync.dma_start(xt[:], xvr[:, :, sc * 512 : (sc + 1) * 512])
                for vh in range(2):
                    psa = mmp.tile([P, 512], dt.float32, tag="mm", name="pva")
                    psb = mmp.tile([P, 512], dt.float32, tag="mm", name="pvb")
                    for dti in range(NDT):
                        for i, psx in enumerate((psa, psb)):
                            sti = vh * 2 + i
                            nc.tensor.matmul(
                                psx[:],
                                lhsT=xt[:, dti, sti * P : (sti + 1) * P],
                                rhs=wvsb[:, dti, :],
                                start=(dti == 0),
                                stop=(dti == NDT - 1),
                            )
                    for i, psx in enumerate((psa, psb)):
                        st = sc * 4 + vh * 2 + i
                        src = psx[:].rearrange("p (h d) -> p h d", d=DH)
                        dstap = VSB[:, st, :, 0:DH]
                        if with_qkv_bias:
                            nc.vector.tensor_tensor(
                                dstap,
                                src,
                                bv_sb[:].rearrange("p (h d) -> p h d", d=DH),
                                ALU.add,
                            )
                        else:
                            nc.vector.tensor_copy(dstap, src)

        # ---------------- attention -----------------
        imr = invm.rearrange("(kt p) q -> p kt q", p=P)
        imqs = {}
        pts = {}

        def load_imq(qc):
            # Quarter tiles (4 k-tiles each) keep the invp pool small while
            # each load is still a single DMA instruction.
            if qc in imqs:
                return
            qsl = slice(qc * 512, (qc + 1) * 512)
            tiles = []
            for k4 in range(4):
                imq = invp.tile([P, 4, 512], dt.bfloat16, tag="im", name="imq")
                nc.sync.dma_start(
                    imq[:], imr[:, k4 * 4 : (k4 + 1) * 4, qsl]
                )
                tiles.append(imq)
            imqs[qc] = tiles

        def emit_scores(qc, hp, pump=None):
            qsl = slice(qc * 512, (qc + 1) * 512)
            load_imq(qc)
            imq = imqs[qc]
            PT = ptp.tile([P, NKT, 2, 512], dt.bfloat16, tag="pt", name="PT")
            pts[(qc, hp)] = PT
            for kt in range(NKT):
                ps = scp.tile([P, 2, 512], dt.float32, tag="sc", name="sc")
                ksl = slice(kt * P, (kt + 1) * P)
                nc.tensor.matmul(
                    ps[:, 0, :],
                    lhsT=KHT[0:DH, hp, ksl],
                    rhs=QHT[0:DH, hp, qsl],
                    start=True,
                    stop=True,
                )
                nc.tensor.matmul(
                    ps[:, 1, :],
                    lhsT=KHT[DH:P, hp, ksl],
                    rhs=QHT[DH:P, hp, qsl],
                    start=True,
                    stop=True,
                )
                nc.scalar.activation(PT[:, kt, :, :], ps[:], AF.Exp)
                if kt % 4 == 3:
                    g = kt // 4
                    for j in range(2):
                        nc.vector.tensor_tensor(
                            PT[:, g * 4 : (g + 1) * 4, j, :],
                            PT[:, g * 4 : (g + 1) * 4, j, :],
                            imq[g][:],
                            ALU.mult,
                        )
                if pump is not None:
                    pump()

        def pv_thunks(qc, hp):
            """Transposed PV + normalize + transpose back into OT, as a list
            of small (fn, pe_cost_ns) thunks for interleaved emission."""
            PT = pts.pop((qc, hp))
            state = {}
            th = []

            def G(qth, qtl, j, alloc):
                def fn():
                    if alloc:
                        state[qth] = mmp.tile(
                            [P, 2, 2, DH + 1], dt.float32, tag="mm", name="pvt"
                        )
                    pv = state[qth]
                    qt = qth * 2 + qtl
                    h = hp * 2 + j
                    for kt in range(NKT):
                        nc.tensor.matmul(
                            pv[:, qtl, j, :],
                            lhsT=PT[:, kt, j, qt * P : (qt + 1) * P],
                            rhs=VSB[:, kt, h, :],
                            start=(kt == 0),
                            stop=(kt == NKT - 1),
                        )
                return fn

            def NT(qth):
                def fn():
                    if qth == 0:
                        state["ott"] = ottp.tile(
                            [P, 4, 2, DH], dt.bfloat16, tag="ott", name="OTT"
                        )
                    OTT = state["ott"]
                    pv = state[qth]
                    tp = mmp.tile([P, 2, P], dt.bfloat16, tag="mm", name="tp")
                    for qtl in range(2):
                        qt = qth * 2 + qtl
                        rc = rcp.tile([P, 2, 1], dt.float32, tag="rc", name="rc")
                        nc.vector.reciprocal(rc[:], pv[:, qtl, :, DH : DH + 1])
                        for j in range(2):
                            nc.vector.tensor_scalar(
                                OTT[:, qt, j, :],
                                pv[:, qtl, j, 0:DH],
                                rc[:, j, :],
                                None,
                                ALU.mult,
                            )
                        nc.tensor.transpose(tp[:, qtl, :], OTT[:, qt, :, :], IDT[:])
                        ssl = slice(qc * 512 + qt * P, qc * 512 + (qt + 1) * P)
                        nc.vector.tensor_copy(OT[:, hp, ssl], tp[:, qtl, :])
                return fn

            for qth in range(2):
                for qtl in range(2):
                    for j in range(2):
                        th.append((G(qth, qtl, j, qtl == 0 and j == 0), 450))
                th.append((NT(qth), 250))
            return th

        def outproj_thunks(qc):
            yr = y.rearrange("(st p) o -> st p o", p=P)
            th = []
            state = {}

            def ST(sti, oc2, cth):
                def fn():
                    st = qc * 4 + sti
                    ssl = slice(st * P, (st + 1) * P)
                    if oc2 == 0 and cth == 0:
                        state[sti] = yp.tile([P, D], dt.bfloat16, tag="y", name="yt")
                    yt = state[sti]
                    osl = slice(oc2 * 512, (oc2 + 1) * 512)
                    if cth == 0:
                        state[(sti, oc2)] = mmp.tile(
                            [P, 512], dt.float32, tag="mm", name="op"
                        )
                    op = state[(sti, oc2)]
                    for ct in range(cth * 2, cth * 2 + 2):
                        nc.tensor.matmul(
                            op[:],
                            lhsT=OT[:, ct, ssl],
                            rhs=WO[:, ct, osl],
                            start=(ct == 0),
                            stop=(ct == NOT - 1),
                        )
                    if cth == 1:
                        nc.vector.tensor_copy(yt[:, osl], op[:])
                        if oc2 == 1:
                            nc.sync.dma_start(yr[st, :, :], yt[:])
                return fn

            for sti in range(4):
                for oc2 in range(2):
                    for cth in range(2):
                        th.append((ST(sti, oc2, cth), 470))
            return th

        def vproj_thunks():
            state = {}
            th = []

            def WDMA():
                state["wv"] = wvp.tile([P, NDT, OC], dt.float16, tag="wv", name="wvsb")
                nc.sync.dma_start(
                    state["wv"][:], wv.rearrange("(dt p) o -> p dt o", p=P)
                )

            def XDMA(sc):
                def fn():
                    xts = []
                    for h in range(2):
                        xh = xpool.tile(
                            [P, NDT // 2, 512], dt.float16, tag="x", name="xtv"
                        )
                        nc.sync.dma_start(
                            xh[:],
                            xvr[:, h * 4 : h * 4 + 4, sc * 512 : (sc + 1) * 512],
                        )
                        xts.append(xh)
                    state["x"] = xts
                return fn

            def VMM(sc, vh, dti):
                # Single-dtile pieces (2 matmuls, ~430ns) so a pop never
                # displaces the score-tile cadence by more than its slack.
                def fn():
                    xt = state["x"][dti // 4]
                    if dti == 0:
                        state[(sc, vh)] = (
                            mmp.tile([P, 512], dt.float32, tag="mm", name="pva"),
                            mmp.tile([P, 512], dt.float32, tag="mm", name="pvb"),
                        )
                    psa, psb = state[(sc, vh)]
                    for i, psx in enumerate((psa, psb)):
                        sti = vh * 2 + i
                        nc.tensor.matmul(
                            psx[:],
                            lhsT=xt[:, dti % 4, sti * P : (sti + 1) * P],
                            rhs=state["wv"][:, dti, :],
                            start=(dti == 0),
                            stop=(dti == NDT - 1),
                        )
                    if dti == NDT - 1:
                        for i, psx in enumerate(state[(sc, vh)]):
                            st = sc * 4 + vh * 2 + i
                            src = psx[:].rearrange("p (h d) -> p h d", d=DH)
                            dstap = VSB[:, st, :, 0:DH]
                            if with_qkv_bias:
                                nc.vector.tensor_tensor(
                                    dstap,
                                    src,
                                    bv_sb[:].rearrange("p (h d) -> p h d", d=DH),
                                    ALU.add,
                                )
                            else:
                                nc.vector.tensor_copy(dstap, src)
                return fn

            th.append((WDMA, 100))
            for sc in range(NQC):
                th.append((XDMA(sc), 100))
                for vh in range(2):
                    for dti in range(NDT):
                        th.append((VMM(sc, vh, dti), 440))
            return th

        def qkproj_thunks(
            xr, w_dram, dst, bias_sb, op, scs=range(NQC), wsb=None, ks=(0, 1)
        ):
            """Deferred projection as fine-grained thunks. Uses its own 1-bank
            PSUM pool (pjp) in 256-column chunks: sharing scp would collapse
            the score-tile double-buffering (exp serializes with the matmuls)
            whenever a proj alloc lands inside the rotation. ``ks`` selects
            one or both o-tiles of the pair (single-o-tile passes let the
            first attention items start before the rest is projected)."""
            state = {}
            th = []
            npieces = 2 * len(ks)  # 4 matmuls (~430ns) per piece

            def WDMA():
                state["w"] = proj_pair_start(w_dram, op)

            def XD(sc):
                def fn():
                    xts = []
                    for h in range(2):
                        xh = xpool.tile(
                            [P, NDT // 2, 512], dt.float16, tag="x", name="xtp"
                        )
                        nc.sync.dma_start(
                            xh[:],
                            xr[:, h * 4 : h * 4 + 4, sc * 512 : (sc + 1) * 512],
                        )
                        xts.append(xh)
                    state["x"] = xts
                return fn

            def PC(sc, scol, piece):
                # piece -> (ki-major, dti-half): each slot's accumulation
                # group fully closes before the next slot's opens.
                def fn():
                    if piece == 0:
                        state["ps"] = pjp.tile(
                            [P, len(ks), 256], dt.float32, tag="pj", name="pjp"
                        )
                    ps = state["ps"]
                    csl = slice(scol * 256, (scol + 1) * 256)
                    ki, half = piece // 2, piece % 2
                    k = ks[ki]
                    for dti in range(half * 4, half * 4 + 4):
                        nc.tensor.matmul(
                            ps[:, ki, :],
                            lhsT=state["w"][:, dti, k * P : (k + 1) * P],
                            rhs=state["x"][dti // 4][:, dti % 4, csl],
                            start=(dti == 0),
                            stop=(dti == NDT - 1),
                        )
                    if half == 1:
                        ot = 2 * op + k
                        dstap = dst[:, ot, sc * 512 + scol * 256 :
                                    sc * 512 + (scol + 1) * 256]
                        if bias_sb is not None:
                            nc.vector.tensor_scalar(
                                dstap,
                                ps[:, ki, :],
                                bias_sb[:, ot : ot + 1],
                                None,
                                ALU.add,
                            )
                        else:
                            nc.vector.tensor_copy(dstap, ps[:, ki, :])
                return fn

            if wsb is not None:
                state["w"] = wsb
            else:
                th.append((WDMA, 100))
            for sc in scs:
                th.append((XD(sc), 100))
                for scol in range(2):
                    for piece in range(npieces):
                        th.append((PC(sc, scol, piece), 440))
            return th

        # ---------------- schedule -----------------
        # Phase A: head pairs 0,1 over all q-chunks; phase B: head pairs 2,3.
        # Required work (V projection, previous item's PV/norm/transpose,
        # output projection) lives in main_q; deferrable pair-1 projections
        # live in fill_q and are popped with spare PE budget or forced just
        # before the phase-B item that needs them. Pops happen between score
        # k-tiles so ScalarE (exp) stays saturated while PE fills its slack.
        from collections import deque

        bqs = bq_sb if with_qkv_bias else None
        bks = bk_sb if with_qkv_bias else None
        order = os.environ.get("BASS_ATTN_ORDER", "phased")
        head = os.environ.get("BASS_ATTN_HEAD", "pair")
        budget_a = int(os.environ.get("BASS_ATTN_BUDGET_A", "450"))
        budget_b = int(os.environ.get("BASS_ATTN_BUDGET_B", "450"))
        budget_cell = [budget_a]
        act_dma = nc.scalar  # Activation engine: second HWDGE queue

        # PE warm-up: the cost model runs the PE at the mid p-state until
        # ~3us of sustained activity; burn that in on junk matmuls while the
        # first weight/x DMAs are still in flight.
        warm = persist.tile([P, 512], dt.bfloat16)
        nc.vector.memset(warm[:], 0.0)
        wps = mmp.tile([P, 512], dt.float32, tag="mm", name="warmps")
        nwarm = int(os.environ.get("BASS_ATTN_WARM", "14"))
        for i in range(nwarm):
            nc.tensor.matmul(
                wps[:], lhsT=warm[:, 0:P], rhs=warm[:], start=(i == 0),
                stop=(i == nwarm - 1),
            )

        def proj_single_sc(xr, wsb, dst, bias_sb, op, k, sc, dma=None):
            """Single o-tile projection chunk (upfront head only; uses scp)."""
            xts = []
            for h in range(2):
                xh = xpool.tile([P, NDT // 2, 512], dt.float16, tag="x", name="xt")
                (dma or nc.sync).dma_start(
                    xh[:], xr[:, h * 4 : h * 4 + 4, sc * 512 : (sc + 1) * 512]
                )
                xts.append(xh)
            ps = scp.tile([P, 2, 512], dt.float32, tag="sc", name="pjs")
            for dti in range(NDT):
                nc.tensor.matmul(
                    ps[:, 0, :],
                    lhsT=wsb[:, dti, k * P : (k + 1) * P],
                    rhs=xts[dti // 4][:, dti % 4, :],
                    start=(dti == 0),
                    stop=(dti == NDT - 1),
                )
            ot = 2 * op + k
            dstap = dst[:, ot, sc * 512 : (sc + 1) * 512]
            if bias_sb is not None:
                nc.vector.tensor_scalar(
                    dstap, ps[:, 0, :], bias_sb[:, ot : ot + 1], None, ALU.add
                )
            else:
                nc.vector.tensor_copy(dstap, ps[:, 0, :])

        # Head: only K o-tile 0 (head pair 0) + the first Q chunk before
        # attention starts; everything else trickles in through the queues.
        wk01 = proj_pair_start(wk, 0)
        wq01 = proj_pair_start(wq, 0)
        if order == "simple":
            for sc in range(NQC):
                proj_pair_sc(xkr, wk01, KHT, bks, 0, sc)
            for sc in range(NQC):
                proj_pair_sc(xqr, wq01, QHT, bqs, 0, sc)
        elif head == "single":
            for sc in range(NQC):
                proj_single_sc(xkr, wk01, KHT, bks, 0, 0, sc, dma=act_dma)
            proj_single_sc(xqr, wq01, QHT, bqs, 0, 0, 0)
        else:
            adma = act_dma if os.environ.get("BASS_ATTN_ACTDMA", "1") == "1" else None
            # K0, K1, Q0, K2, K3: the Q chunk's x DMA fires third (no xpool
            # rotation wait), so scores can start right after K0/K1/Q0 land;
            # score tiles kt4+/kt8+ need K2/K3 which follow just in time.
            proj_pair_sc(xkr, wk01, KHT, bks, 0, 0, dma=adma)
            proj_pair_sc(xkr, wk01, KHT, bks, 0, 1, dma=adma)
            proj_pair_sc(xqr, wq01, QHT, bqs, 0, 0)
            proj_pair_sc(xkr, wk01, KHT, bks, 0, 2, dma=adma)
            proj_pair_sc(xkr, wk01, KHT, bks, 0, 3, dma=adma)
        late_init()

        main_q = deque()   # (fn, cost) or (None, marker_id)
        fill_q = deque()   # (fn, cost) or (None, label)
        credit = [0]

        def pump():
            credit[0] += budget_cell[0]
            while main_q and (main_q[0][0] is None or main_q[0][1] <= credit[0]):
                fn, c = main_q.popleft()
                if fn is not None:
                    fn()
                    credit[0] -= c
            while fill_q and (fill_q[0][0] is None or fill_q[0][1] <= credit[0]):
                fn, c = fill_q.popleft()
                if fn is not None:
                    fn()
                    credit[0] -= c

        def drain_main_to(mk):
            if not any(fn is None and c == mk for fn, c in main_q):
                return
            while main_q:
                fn, c = main_q.popleft()
                if fn is None:
                    if c == mk:
                        return
                    continue
                fn()

        def drain_fill_to(label):
            if not any(fn is None and c == label for fn, c in fill_q):
                return
            while fill_q:
                fn, c = fill_q.popleft()
                if fn is None:
                    if c == label:
                        return
                    continue
                fn()

        def drain_all():
            while main_q:
                fn, c = main_q.popleft()
                if fn is not None:
                    fn()
            while fill_q:
                fn, c = fill_q.popleft()
                if fn is not None:
                    fn()

        def item_thunks(qc, hp):
            th = pv_thunks(qc, hp)
            if hp == NOT - 1:
                # Interleave output-projection pieces so each 128-row block
                # goes out right after its transpose lands.
                ost = outproj_thunks(qc)
                th = th[:5] + ost[0:8] + th[5:] + ost[8:16]
            return th

        if order == "simple":
            wk23 = proj_pair_start(wk, 1)
            for sc in range(NQC):
                proj_pair_sc(xkr, wk23, KHT, bks, 1, sc)
            wq23 = proj_pair_start(wq, 1)
            for sc in range(NQC):
                proj_pair_sc(xqr, wq23, QHT, bqs, 1, sc)
            for fn, c in vproj_thunks():
                fn()
            nc.sync.dma_start(WO[:], wo.rearrange("(ct p) o -> p ct o", p=P))
            items = [(qc, hp) for qc in range(NQC) for hp in range(NOT)]
            for idx in range(len(items)):
                emit_scores(*items[idx])
                for fn, c in item_thunks(*items[idx]):
                    fn()
        else:
            items = [(qc, hp) for qc in range(NQC) for hp in (0, 1)] + [
                (qc, hp) for qc in range(NQC) for hp in (2, 3)
            ]
            main_q.extend(vproj_thunks())
            if head == "single":
                # Deferred: K o-tile 1 (for the hp=1 items), then per-sc Q
                # chunks for both o-tiles.
                fill_q.extend(
                    qkproj_thunks(xkr, wk, KHT, bks, 0, wsb=wk01, ks=(1,))
                )
                fill_q.append((None, "k1"))
                fill_q.extend(
                    qkproj_thunks(
                        xqr, wq, QHT, bqs, 0, scs=(0,), wsb=wq01, ks=(1,)
                    )
                )
                fill_q.append((None, "q1sc0"))
                for sc in range(1, NQC):
                    fill_q.extend(
                        qkproj_thunks(
                            xqr, wq, QHT, bqs, 0, scs=(sc,), wsb=wq01, ks=(0,)
                        )
                    )
                    fill_q.append((None, f"q0sc{sc}"))
                    fill_q.extend(
                        qkproj_thunks(
                            xqr, wq, QHT, bqs, 0, scs=(sc,), wsb=wq01, ks=(1,)
                        )
                    )
                    fill_q.append((None, f"q1sc{sc}"))
            else:
                q01 = qkproj_thunks(
                    xqr, wq, QHT, bqs, 0, scs=range(1, NQC), wsb=wq01
                )
                npc0 = 1 + 8
                for i, sc in enumerate(range(1, NQC)):
                    fill_q.extend(q01[npc0 * i : npc0 * (i + 1)])
                    fill_q.append((None, f"q0sc{sc}"))
                    fill_q.append((None, f"q1sc{sc}"))
            fill_q.extend(qkproj_thunks(xkr, wk, KHT, bks, 1))
            fill_q.append((None, "k23"))
            npc = 1 + 8
            q23 = qkproj_thunks(xqr, wq, QHT, bqs, 1)
            fill_q.append(q23[0])  # weight DMA
            for sc in range(NQC):
                fill_q.extend(q23[1 + npc * sc : 1 + npc * (sc + 1)])
                fill_q.append((None, f"q23sc{sc}"))
            fill_q.append(
                (
                    lambda: nc.sync.dma_start(
                        WO[:], wo.rearrange("(ct p) o -> p ct o", p=P)
                    ),
                    100,
                )
            )
            fill_q.append((None, "wo"))

            nphase = len(items) // 2
            for idx in range(len(items)):
                qc, hp = items[idx]
                # Items 0-1: ScalarE is still working through its initial
                # backlog, so PE can pop aggressively (V projection) without
                # starving exp.
                warm_boost = int(os.environ.get("BASS_ATTN_BUDGET_W", "700"))
                budget_cell[0] = (
                    warm_boost if idx == 1 else
                    budget_a if idx < nphase else budget_b
                )
                if idx >= 2:
                    drain_main_to(idx - 2)
                if idx == nphase:
                    imqs.clear()
                if hp == 0 and qc > 0:
                    drain_fill_to(f"q0sc{qc}")
                if hp == 1:
                    drain_fill_to("k1")
                    drain_fill_to(f"q1sc{qc}")
                if hp == 2:
                    # Phase-B needs pair-1 K fully and Q chunk qc.
                    drain_fill_to("k23")
                    drain_fill_to(f"q23sc{qc}")
                    if idx == nphase:
                        drain_fill_to("wo")
                emit_scores(qc, hp, pump=pump)
                main_q.extend(item_thunks(qc, hp))
                main_q.append((None, idx))  # marker
            drain_all()

    if split_waits:
        _split_multi_waits(nc, mybir)
    return nc


def kernel(q, k, v, mask, wq_w, wq_b, wk_w, wk_b, wv_w, wv_b, wo_w, wo_b):
    global LAST_EXEC_NS, LAST_RESULTS
    from concourse.bass_utils import run_bass_kernel_spmd

    q = np.asarray(q, np.float32)
    k = np.asarray(k, np.float32)
    v = np.asarray(v, np.float32)
    mask = np.asarray(mask)
    wq_w = np.asarray(wq_w, np.float32)
    wk_w = np.asarray(wk_w, np.float32)
    wv_w = np.asarray(wv_w, np.float32)
    wo_w = np.asarray(wo_w, np.float32)
    wq_b = np.asarray(wq_b, np.float32)
    wk_b = np.asarray(wk_b, np.float32)
    wv_b = np.asarray(wv_b, np.float32)
    wo_b = np.asarray(wo_b, np.float32)

    nc, in_maps, with_qkv_bias = prepare(
        q, k, v, mask, wq_w, wq_b, wk_w, wk_b, wv_w, wv_b, wo_w, wo_b
    )

    trace = bool(int(os.environ.get("BASS_ATTN_TRACE", "0")))
    res = run_bass_kernel_spmd(
        nc, in_maps, core_ids=list(range(N_CORES)), trace=trace
    )
    LAST_EXEC_NS = res.exec_time_ns
    LAST_RESULTS = res

    out = np.zeros((B, S, D), np.float32)
    for b in range(B):
        out[b] = (
            res.results[2 * b]["y_part"].astype(np.float32)
            + res.results[2 * b + 1]["y_part"].astype(np.float32)
            + wo_b[None, :]
        )
    return out


def prepare(q, k, v, mask, wq_w, wq_b, wk_w, wk_b, wv_w, wv_b, wo_w, wo_b):
    with_qkv_bias = bool(
        np.asarray(wq_b).any() or np.asarray(wk_b).any() or np.asarray(wv_b).any()
    )
    nc = _build_program(with_qkv_bias)

    ident = np.eye(P, dtype=_BF16)
    in_maps = []
    xts = {}
    invs = {}
    for b in range(B):
        xts[b] = (
            np.ascontiguousarray(q[b].T).astype(np.float16),
            np.ascontiguousarray(k[b].T).astype(np.float16),
            np.ascontiguousarray(v[b].T).astype(np.float16),
        )
        invs[b] = np.ascontiguousarray((~mask[b, 0]).T).astype(_BF16)
    for c in range(N_CORES):
        b, g = c // 2, c % 2
        rows = slice(g * OC, (g + 1) * OC)
        im = {
            "xq_t": xts[b][0],
            "xk_t": xts[b][1],
            "xv_t": xts[b][2],
            "invm_t": invs[b],
            "wq_t": np.ascontiguousarray(wq_w[rows].T).astype(np.float16),
            "wk_t": np.ascontiguousarray(wk_w[rows].T).astype(np.float16),
            "wv_t": np.ascontiguousarray(wv_w[rows].T).astype(np.float16),
            "wo_t": np.ascontiguousarray(wo_w[:, rows].T).astype(_BF16),
            "ident": ident,
        }
        if with_qkv_bias:
            im["bq"] = np.ascontiguousarray(wq_b[rows])
            im["bk"] = np.ascontiguousarray(wk_b[rows])
            im["bv_bcast"] = np.ascontiguousarray(
                np.tile(wv_b[rows][None, :], (P, 1)).astype(np.float32)
            )
        in_maps.append(im)

    return nc, in_maps, with_qkv_bias


# revision 50
# speedup vs baseline: 1.6801x; 1.6801x over previous
"""8-core Trainium2 Bass kernel for nn_Attention_86079734546756.

Sharding: B=4 batches x 2 head-groups (8 heads each) -> 8 cores.
Per core (batch b, head-group g):
  - fp16 Q/K projections (transposed-out, channel-major) emitted in o-tile
    (head-pair) passes so attention on the first head pairs starts early
  - V projection in natural layout with an appended ones column (softmax
    denominators fall out of the PV matmul)
  - scores S^T[k,q] = K_h Q_h^T via fp16 matmuls, two heads packed into the
    128-row PE array (dh=64 contraction each, base_partition 0/64)
  - exp on ScalarE (PSUM->SBUF, bf16 out); mask multiply on VectorE
  - transposed PV: stationary = P^T tile [128k x 128q], moving = V[k, dh|1]
    -> PSUM [q, dh|den]; full 128-deep contraction (half the PE time of the
    natural orientation) and the denominator lands per-PARTITION, so
    normalization is a cheap reciprocal + tensor_scalar on VectorE
  - PE transpose (via host-fed identity) flips [q, c] back to channel-major
    for the output projection
  - output projection (bf16) -> partial y (bf16) summed on host (+ wo_b)
Item order is phased (head-pairs 0,1 for all q-chunks, then 2,3) so exp can
run continuously on ScalarE while remaining projections fill PE gaps.
A post-pass splits multi-wait instructions because this walrus build encodes
one sync wait per instruction.
No max-subtraction in softmax: |alpha| <~ 60 so exp stays in fp32 range,
and masked lanes are zeroed exactly by multiplying with (1-mask) after exp.
"""

import os
import numpy as np
import ml_dtypes

B, S, D, H = 4, 2048, 1024, 16
DH = D // H          # 64
P = 128
HPC = 8              # heads per core
OC = 512             # output features per core (head-group width)
NKT = S // P         # 16 k-tiles
NQC = S // 512       # 4 q-chunks
NOT = OC // P        # 4 o-tiles (head pairs)
NDT = D // P         # 8 d-tiles
N_CORES = 8

LAST_EXEC_NS = None
LAST_RESULTS = None

_BF16 = ml_dtypes.bfloat16


def _split_multi_waits(nc, mybir, max_waits: int = 1):
    """The walrus build in this container encodes at most one sync wait per
    ISA instruction and refuses to split. Move extra waits onto standalone
    EventSemaphore instructions inserted just before, on the same engine —
    the engine executes them in stream order, so semantics are unchanged
    (DMA triggers are simply enqueued after the waits pass)."""
    ctr = 0
    for fn in nc.m.functions:
        for blk in fn.blocks:
            insts = blk.instructions
            if not any(
                inst.sync_info is not None
                and inst.sync_info.on_wait
                and len(inst.sync_info.on_wait) > max_waits
                for inst in insts
            ):
                continue
            out = []
            for inst in insts:
                si = inst.sync_info
                waits = list(si.on_wait) if si is not None and si.on_wait else []
                if len(waits) > max_waits:
                    extra, keep = waits[:-max_waits], waits[-max_waits:]
                    for w in extra:
                        ev = mybir.InstEventSemaphore(
                            name=f"evsplit-{ctr}",
                            engine=inst.engine,
                            ins=[],
                            outs=[],
                            sync_info=mybir.SyncInfo(on_wait=[w], on_update=[]),
                        )
                        ctr += 1
                        out.append(ev)
                    si.on_wait = keep
                out.append(inst)
            blk.instructions = out
    return ctr


def _build_program(with_qkv_bias: bool, split_waits: bool = True):
    from contextlib import ExitStack
    import concourse.bass as bass
    import concourse.mybir as mybir
    import concourse.tile as tile

    dt = mybir.dt
    AF = mybir.ActivationFunctionType
    ALU = mybir.AluOpType

    nc = bass.Bass(trn_type="TRN2")

    xq = nc.declare_dram_parameter("xq_t", [D, S], dt.float16, isOutput=False)
    xk = nc.declare_dram_parameter("xk_t", [D, S], dt.float16, isOutput=False)
    xv = nc.declare_dram_parameter("xv_t", [D, S], dt.float16, isOutput=False)
    invm = nc.declare_dram_parameter("invm_t", [S, S], dt.bfloat16, isOutput=False)
    wq = nc.declare_dram_parameter("wq_t", [D, OC], dt.float16, isOutput=False)
    wk = nc.declare_dram_parameter("wk_t", [D, OC], dt.float16, isOutput=False)
    wv = nc.declare_dram_parameter("wv_t", [D, OC], dt.float16, isOutput=False)
    wo = nc.declare_dram_parameter("wo_t", [OC, D], dt.bfloat16, isOutput=False)
    ident = nc.declare_dram_parameter("ident", [P, P], dt.bfloat16, isOutput=False)
    if with_qkv_bias:
        bq = nc.declare_dram_parameter("bq", [OC], dt.float32, isOutput=False)
        bk = nc.declare_dram_parameter("bk", [OC], dt.float32, isOutput=False)
        bv = nc.declare_dram_parameter("bv_bcast", [P, OC], dt.float32, isOutput=False)
    y = nc.declare_dram_parameter("y_part", [S, D], dt.bfloat16, isOutput=True)

    with tile.TileContext(nc) as tc, ExitStack() as ctx:
        persist = ctx.enter_context(tc.tile_pool(name="persist", bufs=1))
        wqkp = ctx.enter_context(tc.tile_pool(name="wqkp", bufs=2))
        wvp = ctx.enter_context(tc.tile_pool(name="wvp", bufs=1))
        xpool = ctx.enter_context(tc.tile_pool(name="xpool", bufs=6))
        invp = ctx.enter_context(tc.tile_pool(name="invp", bufs=4))
        ptp = ctx.enter_context(tc.tile_pool(name="ptp", bufs=2))
        ottp = ctx.enter_context(tc.tile_pool(name="ottp", bufs=2))
        rcp = ctx.enter_context(tc.tile_pool(name="rcp", bufs=8))
        yp = ctx.enter_context(tc.tile_pool(name="yp", bufs=2))
        scp = ctx.enter_context(tc.tile_pool(name="scp", bufs=2, space="PSUM"))
        mmp = ctx.enter_context(tc.tile_pool(name="mmp", bufs=3, space="PSUM"))
        pjp = ctx.enter_context(tc.tile_pool(name="pjp", bufs=1, space="PSUM"))

        QHT = persist.tile([P, NOT, S], dt.float16)          # [o%128, ot, s]
        KHT = persist.tile([P, NOT, S], dt.float16)
        VSB = persist.tile([P, NKT, HPC, DH + 1], dt.bfloat16)  # [k%128, kt, h, dh|1]
        OT = persist.tile([P, NOT, S], dt.bfloat16)          # [c%128, ct, s]
        WO = persist.tile([P, NOT, D], dt.bfloat16)          # [c%128, ct, o]
        IDT = persist.tile([P, P], dt.bfloat16)

        def late_init():
            # Deferred so the first projection's weight/x DMAs go out first.
            nc.sync.dma_start(IDT[:], ident[:])
            nc.vector.memset(VSB[:, :, :, DH : DH + 1], 1.0)

        if with_qkv_bias:
            bq_sb = persist.tile([P, NOT], dt.float32)
            nc.sync.dma_start(bq_sb[:], bq.rearrange("(ot p) -> p ot", p=P))
            bk_sb = persist.tile([P, NOT], dt.float32)
            nc.sync.dma_start(bk_sb[:], bk.rearrange("(ot p) -> p ot", p=P))
            bv_sb = persist.tile([P, OC], dt.float32)
            nc.sync.dma_start(bv_sb[:], bv[:])

        # ---------------- projections -----------------
        xqr = xq.rearrange("(dt p) s -> p dt s", p=P)
        xkr = xk.rearrange("(dt p) s -> p dt s", p=P)
        xvr = xv.rearrange("(dt p) s -> p dt s", p=P)

        def proj_pair_start(w_dram, op):
            """DMA the weights for o-tile pair (2*op, 2*op+1)."""
            wsb = wqkp.tile([P, NDT, 2 * P], dt.float16, tag="wqk", name="wsb")
            wr = w_dram.rearrange("(dt p) o -> p dt o", p=P)
            nc.sync.dma_start(wsb[:], wr[:, :, 2 * op * P : (2 * op + 2) * P])
            return wsb

        def proj_pair_sc(xr, wsb, dst, bias_sb, op, sc, dma=None):
            """One 512-token chunk of the projection for o-tile pair op.
            Loads all 8 d-tiles of x in a single DMA (the SP sequencer costs
            ~565ns per DMA instruction, so small per-dtile DMAs starve PE)."""
            # x loads in two 4-dtile halves so DMA triggers pipeline finely
            # (one 8-dtile load per chunk left the PE idle waiting the tail
            # of a 1MB transfer, which also resets the PE p-state). Alloc
            # PSUM after the DMAs: the scp-slot wait must not attach to the
            # DMA trigger.
            sl = slice(sc * 512, (sc + 1) * 512)
            xts = []
            for h in range(2):
                xh = xpool.tile([P, NDT // 2, 512], dt.float16, tag="x", name="xt")
                (dma or nc.sync).dma_start(xh[:], xr[:, h * 4 : h * 4 + 4, sl])
                xts.append(xh)
            ps = scp.tile([P, 2, 512], dt.float32, tag="sc", name="pjp")
            # k outer: one PSUM accumulation group fully closes before the
            # next slot's group opens (interleaved open groups in one tile
            # are rejected by the simulator's zero-region model).
            for k in range(2):
                for dti in range(NDT):
                    nc.tensor.matmul(
                        ps[:, k, :],
                        lhsT=wsb[:, dti, k * P : (k + 1) * P],
                        rhs=xts[dti // 4][:, dti % 4, :],
                        start=(dti == 0),
                        stop=(dti == NDT - 1),
                    )
            for k in range(2):
                ot = 2 * op + k
                dstap = dst[:, ot, sc * 512 : (sc + 1) * 512]
                if bias_sb is not None:
                    nc.vector.tensor_scalar(
                        dstap, ps[:, k, :], bias_sb[:, ot : ot + 1], None, ALU.add
                    )
                else:
                    nc.vector.tensor_copy(dstap, ps[:, k, :])

        def proj_pair(xr, w_dram, dst, bias_sb, op):
            wsb = proj_pair_start(w_dram, op)
            for sc in range(NQC):
                proj_pair_sc(xr, wsb, dst, bias_sb, op, sc)

        def emit_vproj():
            # V: natural layout [s, o] scattered into VSB[k%128, kt, h, 0:64].
            wvsb = wvp.tile([P, NDT, OC], dt.float16, tag="wv", name="wvsb")
            nc.sync.dma_start(wvsb[:], wv.rearrange("(dt p) o -> p dt o", p=P))
            for sc in range(NQC):
                xt = xpool.tile([P, NDT, 512], dt.float16, tag="x", name="xtv")
                nc.sync.dma_start(xt[:], xvr[:, :, sc * 512 : (sc + 1) * 512])
                for vh in range(2):
                    psa = mmp.tile([P, 512], dt.float32, tag="mm", name="pva")
                    psb = mmp.tile([P, 512], dt.float32, tag="mm", name="pvb")
                    for dti in range(NDT):
                        for i, psx in enumerate((psa, psb)):
                            sti = vh * 2 + i
                            nc.tensor.matmul(
                                psx[:],
                                lhsT=xt[:, dti, sti * P : (sti + 1) * P],
                                rhs=wvsb[:, dti, :],
                                start=(dti == 0),
                                stop=(dti == NDT - 1),
                            )
                    for i, psx in enumerate((psa, psb)):
                        st = sc * 4 + vh * 2 + i
                        src = psx[:].rearrange("p (h d) -> p h d", d=DH)
                        dstap = VSB[:, st, :, 0:DH]
                        if with_qkv_bias:
                            nc.vector.tensor_tensor(
                                dstap,
                                src,
                                bv_sb[:].rearrange("p (h d) -> p h d", d=DH),
                                ALU.add,
                            )
                        else:
                            nc.vector.tensor_copy(dstap, src)

        # ---------------- attention -----------------
        imr = invm.rearrange("(kt p) q -> p kt q", p=P)
        imqs = {}
        pts = {}

        def load_imq(qc):
            # Quarter tiles (4 k-tiles each) keep the invp pool small while
            # each load is still a single DMA instruction.
            if qc in imqs:
                return
            qsl = slice(qc * 512, (qc + 1) * 512)
            tiles = []
            for k4 in range(4):
                imq = invp.tile([P, 4, 512], dt.bfloat16, tag="im", name="imq")
                nc.sync.dma_start(
                    imq[:], imr[:, k4 * 4 : (k4 + 1) * 4, qsl]
                )
                tiles.append(imq)
            imqs[qc] = tiles

        def emit_scores(qc, hp, pump=None, kt_hook=None):
            qsl = slice(qc * 512, (qc + 1) * 512)
            load_imq(qc)
            imq = imqs[qc]
            PT = ptp.tile([P, NKT, 2, 512], dt.bfloat16, tag="pt", name="PT")
            pts[(qc, hp)] = PT
            for kt in range(NKT):
                if kt_hook is not None:
                    kt_hook(kt)
                ps = scp.tile([P, 2, 512], dt.float32, tag="sc", name="sc")
                ksl = slice(kt * P, (kt + 1) * P)
                nc.tensor.matmul(
                    ps[:, 0, :],
                    lhsT=KHT[0:DH, hp, ksl],
                    rhs=QHT[0:DH, hp, qsl],
                    start=True,
                    stop=True,
                )
                nc.tensor.matmul(
                    ps[:, 1, :],
                    lhsT=KHT[DH:P, hp, ksl],
                    rhs=QHT[DH:P, hp, qsl],
                    start=True,
                    stop=True,
                )
                nc.scalar.activation(PT[:, kt, :, :], ps[:], AF.Exp)
                if kt % 4 == 3:
                    g = kt // 4
                    for j in range(2):
                        nc.vector.tensor_tensor(
                            PT[:, g * 4 : (g + 1) * 4, j, :],
                            PT[:, g * 4 : (g + 1) * 4, j, :],
                            imq[g][:],
                            ALU.mult,
                        )
                if pump is not None:
                    pump()

        def pv_thunks(qc, hp):
            """Transposed PV + normalize + transpose back into OT, as a list
            of small (fn, pe_cost_ns) thunks for interleaved emission."""
            PT = pts.pop((qc, hp))
            state = {}
            th = []

            def G(qth, qtl, j, alloc):
                def fn():
                    if alloc:
                        state[qth] = mmp.tile(
                            [P, 2, 2, DH + 1], dt.float32, tag="mm", name="pvt"
                        )
                    pv = state[qth]
                    qt = qth * 2 + qtl
                    h = hp * 2 + j
                    for kt in range(NKT):
                        nc.tensor.matmul(
                            pv[:, qtl, j, :],
                            lhsT=PT[:, kt, j, qt * P : (qt + 1) * P],
                            rhs=VSB[:, kt, h, :],
                            start=(kt == 0),
                            stop=(kt == NKT - 1),
                        )
                return fn

            def NT(qth):
                def fn():
                    if qth == 0:
                        state["ott"] = ottp.tile(
                            [P, 4, 2, DH], dt.bfloat16, tag="ott", name="OTT"
                        )
                    OTT = state["ott"]
                    pv = state[qth]
                    tp = mmp.tile([P, 2, P], dt.bfloat16, tag="mm", name="tp")
                    for qtl in range(2):
                        qt = qth * 2 + qtl
                        rc = rcp.tile([P, 2, 1], dt.float32, tag="rc", name="rc")
                        nc.vector.reciprocal(rc[:], pv[:, qtl, :, DH : DH + 1])
                        for j in range(2):
                            nc.vector.tensor_scalar(
                                OTT[:, qt, j, :],
                                pv[:, qtl, j, 0:DH],
                                rc[:, j, :],
                                None,
                                ALU.mult,
                            )
                        nc.tensor.transpose(tp[:, qtl, :], OTT[:, qt, :, :], IDT[:])
                        ssl = slice(qc * 512 + qt * P, qc * 512 + (qt + 1) * P)
                        nc.vector.tensor_copy(OT[:, hp, ssl], tp[:, qtl, :])
                return fn

            for qth in range(2):
                for qtl in range(2):
                    for j in range(2):
                        th.append((G(qth, qtl, j, qtl == 0 and j == 0), 450))
                th.append((NT(qth), 250))
            return th

        def outproj_thunks(qc):
            yr = y.rearrange("(st p) o -> st p o", p=P)
            th = []
            state = {}

            def ST(sti, oc2, cth):
                def fn():
                    st = qc * 4 + sti
                    ssl = slice(st * P, (st + 1) * P)
                    if oc2 == 0 and cth == 0:
                        state[sti] = yp.tile([P, D], dt.bfloat16, tag="y", name="yt")
                    yt = state[sti]
                    osl = slice(oc2 * 512, (oc2 + 1) * 512)
                    if cth == 0:
                        state[(sti, oc2)] = mmp.tile(
                            [P, 512], dt.float32, tag="mm", name="op"
                        )
                    op = state[(sti, oc2)]
                    for ct in range(cth * 2, cth * 2 + 2):
                        nc.tensor.matmul(
                            op[:],
                            lhsT=OT[:, ct, ssl],
                            rhs=WO[:, ct, osl],
                            start=(ct == 0),
                            stop=(ct == NOT - 1),
                        )
                    if cth == 1:
                        nc.vector.tensor_copy(yt[:, osl], op[:])
                        if oc2 == 1:
                            nc.sync.dma_start(yr[st, :, :], yt[:])
                return fn

            for sti in range(4):
                for oc2 in range(2):
                    for cth in range(2):
                        th.append((ST(sti, oc2, cth), 470))
            return th

        def vproj_thunks():
            state = {}
            th = []

            def WDMA():
                state["wv"] = wvp.tile([P, NDT, OC], dt.float16, tag="wv", name="wvsb")
                nc.sync.dma_start(
                    state["wv"][:], wv.rearrange("(dt p) o -> p dt o", p=P)
                )

            def XDMA(sc):
                def fn():
                    xts = []
                    for h in range(2):
                        xh = xpool.tile(
                            [P, NDT // 2, 512], dt.float16, tag="x", name="xtv"
                        )
                        nc.sync.dma_start(
                            xh[:],
                            xvr[:, h * 4 : h * 4 + 4, sc * 512 : (sc + 1) * 512],
                        )
                        xts.append(xh)
                    state["x"] = xts
                return fn

            def VMM(sc, vh, dti):
                # Single-dtile pieces (2 matmuls, ~430ns) so a pop never
                # displaces the score-tile cadence by more than its slack.
                def fn():
                    xt = state["x"][dti // 4]
                    if dti == 0:
                        state[(sc, vh)] = (
                            mmp.tile([P, 512], dt.float32, tag="mm", name="pva"),
                            mmp.tile([P, 512], dt.float32, tag="mm", name="pvb"),
                        )
                    psa, psb = state[(sc, vh)]
                    for i, psx in enumerate((psa, psb)):
                        sti = vh * 2 + i
                        nc.tensor.matmul(
                            psx[:],
                            lhsT=xt[:, dti % 4, sti * P : (sti + 1) * P],
                            rhs=state["wv"][:, dti, :],
                            start=(dti == 0),
                            stop=(dti == NDT - 1),
                        )
                    if dti == NDT - 1:
                        for i, psx in enumerate(state[(sc, vh)]):
                            st = sc * 4 + vh * 2 + i
                            src = psx[:].rearrange("p (h d) -> p h d", d=DH)
                            dstap = VSB[:, st, :, 0:DH]
                            if with_qkv_bias:
                                nc.vector.tensor_tensor(
                                    dstap,
                                    src,
                                    bv_sb[:].rearrange("p (h d) -> p h d", d=DH),
                                    ALU.add,
                                )
                            else:
                                nc.vector.tensor_copy(dstap, src)
                return fn

            th.append((WDMA, 100))
            for sc in range(NQC):
                th.append((XDMA(sc), 100))
                for vh in range(2):
                    for dti in range(NDT):
                        th.append((VMM(sc, vh, dti), 440))
            return th

        def qkproj_thunks(
            xr, w_dram, dst, bias_sb, op, scs=range(NQC), wsb=None, ks=(0, 1)
        ):
            """Deferred projection as fine-grained thunks. Uses its own 1-bank
            PSUM pool (pjp) in 256-column chunks: sharing scp would collapse
            the score-tile double-buffering (exp serializes with the matmuls)
            whenever a proj alloc lands inside the rotation. ``ks`` selects
            one or both o-tiles of the pair (single-o-tile passes let the
            first attention items start before the rest is projected)."""
            state = {}
            th = []
            npieces = 2 * len(ks)  # 4 matmuls (~430ns) per piece

            def WDMA():
                state["w"] = proj_pair_start(w_dram, op)

            def XD(sc):
                def fn():
                    xts = []
                    for h in range(2):
                        xh = xpool.tile(
                            [P, NDT // 2, 512], dt.float16, tag="x", name="xtp"
                        )
                        nc.sync.dma_start(
                            xh[:],
                            xr[:, h * 4 : h * 4 + 4, sc * 512 : (sc + 1) * 512],
                        )
                        xts.append(xh)
                    state["x"] = xts
                return fn

            def PC(sc, scol, piece):
                # piece -> (ki-major, dti-half): each slot's accumulation
                # group fully closes before the next slot's opens.
                def fn():
                    if piece == 0:
                        state["ps"] = pjp.tile(
                            [P, len(ks), 256], dt.float32, tag="pj", name="pjp"
                        )
                    ps = state["ps"]
                    csl = slice(scol * 256, (scol + 1) * 256)
                    ki, half = piece // 2, piece % 2
                    k = ks[ki]
                    for dti in range(half * 4, half * 4 + 4):
                        nc.tensor.matmul(
                            ps[:, ki, :],
                            lhsT=state["w"][:, dti, k * P : (k + 1) * P],
                            rhs=state["x"][dti // 4][:, dti % 4, csl],
                            start=(dti == 0),
                            stop=(dti == NDT - 1),
                        )
                    if half == 1:
                        ot = 2 * op + k
                        dstap = dst[:, ot, sc * 512 + scol * 256 :
                                    sc * 512 + (scol + 1) * 256]
                        if bias_sb is not None:
                            nc.vector.tensor_scalar(
                                dstap,
                                ps[:, ki, :],
                                bias_sb[:, ot : ot + 1],
                                None,
                                ALU.add,
                            )
                        else:
                            nc.vector.tensor_copy(dstap, ps[:, ki, :])
                return fn

            if wsb is not None:
                state["w"] = wsb
            else:
                th.append((WDMA, 100))
            for sc in scs:
                th.append((XD(sc), 100))
                for scol in range(2):
                    for piece in range(npieces):
                        th.append((PC(sc, scol, piece), 440))
            return th

        # ---------------- schedule -----------------
        # Phase A: head pairs 0,1 over all q-chunks; phase B: head pairs 2,3.
        # Required work (V projection, previous item's PV/norm/transpose,
        # output projection) lives in main_q; deferrable pair-1 projections
        # live in fill_q and are popped with spare PE budget or forced just
        # before the phase-B item that needs them. Pops happen between score
        # k-tiles so ScalarE (exp) stays saturated while PE fills its slack.
        from collections import deque

        bqs = bq_sb if with_qkv_bias else None
        bks = bk_sb if with_qkv_bias else None
        order = os.environ.get("BASS_ATTN_ORDER", "phased")
        head = os.environ.get("BASS_ATTN_HEAD", "pair")
        budget_a = int(os.environ.get("BASS_ATTN_BUDGET_A", "450"))
        budget_b = int(os.environ.get("BASS_ATTN_BUDGET_B", "450"))
        budget_cell = [budget_a]
        act_dma = nc.scalar  # Activation engine: second HWDGE queue

        # PE warm-up: the cost model runs the PE at the mid p-state until
        # ~3us of sustained activity; burn that in on junk matmuls while the
        # first weight/x DMAs are still in flight.
        warm = persist.tile([P, 512], dt.bfloat16)
        nc.vector.memset(warm[:], 0.0)
        wps = mmp.tile([P, 512], dt.float32, tag="mm", name="warmps")
        nwarm = int(os.environ.get("BASS_ATTN_WARM", "14"))
        for i in range(nwarm):
            nc.tensor.matmul(
                wps[:], lhsT=warm[:, 0:P], rhs=warm[:], start=(i == 0),
                stop=(i == nwarm - 1),
            )

        def proj_single_sc(xr, wsb, dst, bias_sb, op, k, sc, dma=None):
            """Single o-tile projection chunk (upfront head only; uses scp)."""
            xts = []
            for h in range(2):
                xh = xpool.tile([P, NDT // 2, 512], dt.float16, tag="x", name="xt")
                (dma or nc.sync).dma_start(
                    xh[:], xr[:, h * 4 : h * 4 + 4, sc * 512 : (sc + 1) * 512]
                )
                xts.append(xh)
            ps = scp.tile([P, 2, 512], dt.float32, tag="sc", name="pjs")
            for dti in range(NDT):
                nc.tensor.matmul(
                    ps[:, 0, :],
                    lhsT=wsb[:, dti, k * P : (k + 1) * P],
                    rhs=xts[dti // 4][:, dti % 4, :],
                    start=(dti == 0),
                    stop=(dti == NDT - 1),
                )
            ot = 2 * op + k
            dstap = dst[:, ot, sc * 512 : (sc + 1) * 512]
            if bias_sb is not None:
                nc.vector.tensor_scalar(
                    dstap, ps[:, 0, :], bias_sb[:, ot : ot + 1], None, ALU.add
                )
            else:
                nc.vector.tensor_copy(dstap, ps[:, 0, :])

        # Head: only K o-tile 0 (head pair 0) + the first Q chunk before
        # attention starts; everything else trickles in through the queues.
        wk01 = proj_pair_start(wk, 0)
        wq01 = proj_pair_start(wq, 0)
        if order == "simple":
            for sc in range(NQC):
                proj_pair_sc(xkr, wk01, KHT, bks, 0, sc)
            for sc in range(NQC):
                proj_pair_sc(xqr, wq01, QHT, bqs, 0, sc)
        elif head == "single":
            for sc in range(NQC):
                proj_single_sc(xkr, wk01, KHT, bks, 0, 0, sc, dma=act_dma)
            proj_single_sc(xqr, wq01, QHT, bqs, 0, 0, 0)
        else:
            adma = act_dma if os.environ.get("BASS_ATTN_ACTDMA", "1") == "1" else None
            # K0, K1, Q0, K2, K3: the Q chunk's x DMA fires third (no xpool
            # rotation wait). Just-in-time K chunks via kt hooks were tried
            # (first exp 30us -> 16us) but the total regressed: fill work is
            # conserved, the idle just moved into the mid-phase PE deficit.
            proj_pair_sc(xkr, wk01, KHT, bks, 0, 0, dma=adma)
            proj_pair_sc(xkr, wk01, KHT, bks, 0, 1, dma=adma)
            proj_pair_sc(xqr, wq01, QHT, bqs, 0, 0)
            proj_pair_sc(xkr, wk01, KHT, bks, 0, 2, dma=adma)
            proj_pair_sc(xkr, wk01, KHT, bks, 0, 3, dma=adma)
        late_init()

        main_q = deque()   # (fn, cost) or (None, marker_id)
        fill_q = deque()   # (fn, cost) or (None, label)
        credit = [0]

        def pump():
            credit[0] += budget_cell[0]
            while main_q and (main_q[0][0] is None or main_q[0][1] <= credit[0]):
                fn, c = main_q.popleft()
                if fn is not None:
                    fn()
                    credit[0] -= c
            while fill_q and (fill_q[0][0] is None or fill_q[0][1] <= credit[0]):
                fn, c = fill_q.popleft()
                if fn is not None:
                    fn()
                    credit[0] -= c

        def drain_main_to(mk):
            if not any(fn is None and c == mk for fn, c in main_q):
                return
            while main_q:
                fn, c = main_q.popleft()
                if fn is None:
                    if c == mk:
                        return
                    continue
                fn()

        def drain_fill_to(label):
            if not any(fn is None and c == label for fn, c in fill_q):
                return
            while fill_q:
                fn, c = fill_q.popleft()
                if fn is None:
                    if c == label:
                        return
                    continue
                fn()

        def drain_all():
            while main_q:
                fn, c = main_q.popleft()
                if fn is not None:
                    fn()
            while fill_q:
                fn, c = fill_q.popleft()
                if fn is not None:
                    fn()

        def item_thunks(qc, hp):
            th = pv_thunks(qc, hp)
            if hp == NOT - 1:
                # Interleave output-projection pieces so each 128-row block
                # goes out right after its transpose lands.
                ost = outproj_thunks(qc)
                th = th[:5] + ost[0:8] + th[5:] + ost[8:16]
            return th

        if order == "simple":
            wk23 = proj_pair_start(wk, 1)
            for sc in range(NQC):
                proj_pair_sc(xkr, wk23, KHT, bks, 1, sc)
            wq23 = proj_pair_start(wq, 1)
            for sc in range(NQC):
                proj_pair_sc(xqr, wq23, QHT, bqs, 1, sc)
            for fn, c in vproj_thunks():
                fn()
            nc.sync.dma_start(WO[:], wo.rearrange("(ct p) o -> p ct o", p=P))
            items = [(qc, hp) for qc in range(NQC) for hp in range(NOT)]
            for idx in range(len(items)):
                emit_scores(*items[idx])
                for fn, c in item_thunks(*items[idx]):
                    fn()
        else:
            items = [(qc, hp) for qc in range(NQC) for hp in (0, 1)] + [
                (qc, hp) for qc in range(NQC) for hp in (2, 3)
            ]
            main_q.extend(vproj_thunks())
            if head == "single":
                # Deferred: K o-tile 1 (for the hp=1 items), then per-sc Q
                # chunks for both o-tiles.
                fill_q.extend(
                    qkproj_thunks(xkr, wk, KHT, bks, 0, wsb=wk01, ks=(1,))
                )
                fill_q.append((None, "k1"))
                fill_q.extend(
                    qkproj_thunks(
                        xqr, wq, QHT, bqs, 0, scs=(0,), wsb=wq01, ks=(1,)
                    )
                )
                fill_q.append((None, "q1sc0"))
                for sc in range(1, NQC):
                    fill_q.extend(
                        qkproj_thunks(
                            xqr, wq, QHT, bqs, 0, scs=(sc,), wsb=wq01, ks=(0,)
                        )
                    )
                    fill_q.append((None, f"q0sc{sc}"))
                    fill_q.extend(
                        qkproj_thunks(
                            xqr, wq, QHT, bqs, 0, scs=(sc,), wsb=wq01, ks=(1,)
                        )
                    )
                    fill_q.append((None, f"q1sc{sc}"))
            else:
                q01 = qkproj_thunks(
                    xqr, wq, QHT, bqs, 0, scs=range(1, NQC), wsb=wq01
                )
                npc0 = 1 + 8
                for i, sc in enumerate(range(1, NQC)):
                    fill_q.extend(q01[npc0 * i : npc0 * (i + 1)])
                    fill_q.append((None, f"q0sc{sc}"))
                    fill_q.append((None, f"q1sc{sc}"))
            fill_q.extend(qkproj_thunks(xkr, wk, KHT, bks, 1))
            fill_q.append((None, "k23"))
            npc = 1 + 8
            q23 = qkproj_thunks(xqr, wq, QHT, bqs, 1)
            fill_q.append(q23[0])  # weight DMA
            for sc in range(NQC):
                fill_q.extend(q23[1 + npc * sc : 1 + npc * (sc + 1)])
                fill_q.append((None, f"q23sc{sc}"))
            fill_q.append(
                (
                    lambda: nc.sync.dma_start(
                        WO[:], wo.rearrange("(ct p) o -> p ct o", p=P)
                    ),
                    100,
                )
            )
            fill_q.append((None, "wo"))

            nphase = len(items) // 2
            for idx in range(len(items)):
                qc, hp = items[idx]
                # Items 0-1: ScalarE is still working through its initial
                # backlog, so PE can pop aggressively (V projection) without
                # starving exp.
                warm_boost = int(os.environ.get("BASS_ATTN_BUDGET_W", "700"))
                ob = int(os.environ.get("BASS_ATTN_BUDGET_O", "0")) or budget_b
                budget_cell[0] = (
                    warm_boost if idx == 1 else
                    budget_a if idx < nphase else
                    (ob if hp == NOT - 1 else budget_b)
                )
                if idx >= 2:
                    drain_main_to(idx - 2)
                if idx == nphase:
                    imqs.clear()
                if hp == 0 and qc > 0:
                    drain_fill_to(f"q0sc{qc}")
                if hp == 1:
                    drain_fill_to("k1")
                    drain_fill_to(f"q1sc{qc}")
                if hp == 2:
                    # Phase-B needs pair-1 K fully and Q chunk qc.
                    drain_fill_to("k23")
                    drain_fill_to(f"q23sc{qc}")
                    if idx == nphase:
                        drain_fill_to("wo")
                emit_scores(qc, hp, pump=pump)
                main_q.extend(item_thunks(qc, hp))
                main_q.append((None, idx))  # marker
            drain_all()

    if split_waits:
        _split_multi_waits(nc, mybir)
    return nc


def kernel(q, k, v, mask, wq_w, wq_b, wk_w, wk_b, wv_w, wv_b, wo_w, wo_b):
    global LAST_EXEC_NS, LAST_RESULTS
    from concourse.bass_utils import run_bass_kernel_spmd

    q = np.asarray(q, np.float32)
    k = np.asarray(k, np.float32)
    v = np.asarray(v, np.float32)
    mask = np.asarray(mask)
    wq_w = np.asarray(wq_w, np.float32)
    wk_w = np.asarray(wk_w, np.float32)
    wv_w = np.asarray(wv_w, np.float32)
    wo_w = np.asarray(wo_w, np.float32)
    wq_b = np.asarray(wq_b, np.float32)
    wk_b = np.asarray(wk_b, np.float32)
    wv_b = np.asarray(wv_b, np.float32)
    wo_b = np.asarray(wo_b, np.float32)

    nc, in_maps, with_qkv_bias = prepare(
        q, k, v, mask, wq_w, wq_b, wk_w, wk_b, wv_w, wv_b, wo_w, wo_b
    )

    trace = bool(int(os.environ.get("BASS_ATTN_TRACE", "0")))
    res = run_bass_kernel_spmd(
        nc, in_maps, core_ids=list(range(N_CORES)), trace=trace
    )
    LAST_EXEC_NS = res.exec_time_ns
    LAST_RESULTS = res

    out = np.zeros((B, S, D), np.float32)
    for b in range(B):
        out[b] = (
            res.results[2 * b]["y_part"].astype(np.float32)
            + res.results[2 * b + 1]["y_part"].astype(np.float32)
            + wo_b[None, :]
        )
    return out


def prepare(q, k, v, mask, wq_w, wq_b, wk_w, wk_b, wv_w, wv_b, wo_w, wo_b):
    with_qkv_bias = bool(
        np.asarray(wq_b).any() or np.asarray(wk_b).any() or np.asarray(wv_b).any()
    )
    nc = _build_program(with_qkv_bias)

    ident = np.eye(P, dtype=_BF16)
    in_maps = []
    xts = {}
    invs = {}
    for b in range(B):
        xts[b] = (
            np.ascontiguousarray(q[b].T).astype(np.float16),
            np.ascontiguousarray(k[b].T).astype(np.float16),
            np.ascontiguousarray(v[b].T).astype(np.float16),
        )
        invs[b] = np.ascontiguousarray((~mask[b, 0]).T).astype(_BF16)
    for c in range(N_CORES):
        b, g = c // 2, c % 2
        rows = slice(g * OC, (g + 1) * OC)
        im = {
            "xq_t": xts[b][0],
            "xk_t": xts[b][1],
            "xv_t": xts[b][2],
            "invm_t": invs[b],
            "wq_t": np.ascontiguousarray(wq_w[rows].T).astype(np.float16),
            "wk_t": np.ascontiguousarray(wk_w[rows].T).astype(np.float16),
            "wv_t": np.ascontiguousarray(wv_w[rows].T).astype(np.float16),
            "wo_t": np.ascontiguousarray(wo_w[:, rows].T).astype(_BF16),
            "ident": ident,
        }
        if with_qkv_bias:
            im["bq"] = np.ascontiguousarray(wq_b[rows])
            im["bk"] = np.ascontiguousarray(wk_b[rows])
            im["bv_bcast"] = np.ascontiguousarray(
                np.tile(wv_b[rows][None, :], (P, 1)).astype(np.float32)
            )
        in_maps.append(im)

    return nc, in_maps, with_qkv_bias
